# revision 47
# baseline (speedup 1.0000x reference)
"""Trainium2 Bass kernel for the quirky-reshape 16-head attention layer.

Shapes (hardcoded): x [2, 2048, 1024], Wq/Wk/Wv/Wo [1024, 1024], n_head=16.

Sharding: core c in [0,8) handles batch b=c//4 and head group g=c%4 (heads
4g..4g+3). The reference's quirky `qkv.reshape(b, s, d)` merge makes output
rows [h*128, (h+1)*128) depend only on head h, so each core produces the
disjoint output row block [g*512, (g+1)*512) of its batch - no collectives.

Precision: q/k path (projections + scores) in fp16, exp / AV / O-projection
in bf16 (fp32 range needed: exp values reach ~1e30), all matmul accumulation
in fp32 PSUM.

Per-core dataflow (transposed-scores streaming attention):
  qT/kT = W^T x^T [128, 2048] fp16 pair tiles (2 heads x 64 rows).  Score
  matmuls are K=64 per head, row-tiled via base_partition 0 / 64
  (tile_position (0,0)/(64,0)).  Measured: row-tiled pairs do NOT overlap
  beyond the normal pipeline drain (same ~225ns/512-col interval as K=128),
  but K=64 with both row groups active keeps the HAM clock at 2.4 GHz and
  drops the zero-padded kTz tiles, memsets and half-copies of the K=128 form.
  V[kb] [128, 4, 128] bf16: per-head blocks [ones(64) | v(64)].
  per (head-pair, 1024-wide q chunk), streaming over 16 key blocks kb:
    S^T[kb,hl] = kT_hl[:,kb]^T qT_hl   -> PSUM [128, 1024] fp32 (x2 heads)
    E[kb,hl]   = exp(S^T)              -> ScalarE, SBUF bf16 (paces the loop)
    AV[hl,half] += [1|v]^T E[kb-1]     -> PSUM [128, 512] x4, one kb behind
  AV rows 0:64 = softmax denominator broadcast, rows 64:128 = qkv unnorm.
  rcp = reciprocal_approx_fast(denom); DMA rcp to partitions 64:128;
  q2[64:128, hg, r, j] = qkv * rcp (bf16, contiguous (r,j) writes);
  q2[0:64, hg, :, odd j] = DMA shift of even j.
  out_h [128, 1024] = sum_kt q2[:, h, :, 2kt+1]^T Wo[kt]  (strided stationary).
  Projection groups run as PSUM-slot-sized fillers inside the attention
  loops, v groups just in time for their AV consumer; pair-0 O-projection
  halves trail attn1 (covering the PE while the tail rcp/mul/shift chain
  drains) and pair-1 runs as a dense tail on the freed av-tag PSUM banks.

Schedule notes (measured on HW): PE is the binding engine (~201us busy of
~232us span; ACT exp ~154us at 66%).  Input DMA: only sync/scalar are
hardware-DGE (~141GB/s each, ~9us ring bring-up; gpsimd is ~22GB/s SWDGE),
so the startup-critical prefix is trimmed to ~3MB (pair-0 weight slices +
seq-low xt halves) and Wo/xt7 ride the queue tails.  PSUM budget: sc-tag
2x[128,1024] (score double-buffer, shared by filler draws) + av-tag
4x[128,512] (AV accumulators) = all 8 banks.
"""

import numpy as np

B, S, D, H = 2, 2048, 1024, 16
DH = 64
NCORES = 8

_CACHE = {}


def _build_program():
    from concourse import bacc, tile, mybir

    F32 = mybir.dt.float32
    F16 = mybir.dt.float16
    BF16 = mybir.dt.bfloat16
    EXP = mybir.ActivationFunctionType.Exp

    nc = bacc.Bacc(None, target_bir_lowering=False, debug=False)

    xt_d = nc.dram_tensor("xt", [128, 8, 2048], F16, kind="ExternalInput").ap()
    wq_d = nc.dram_tensor("wq", [128, 8, 256], F16, kind="ExternalInput").ap()
    wk_d = nc.dram_tensor("wk", [128, 8, 256], F16, kind="ExternalInput").ap()
    wv_d = nc.dram_tensor("wv", [128, 8, 256], F16, kind="ExternalInput").ap()
    wo_d = nc.dram_tensor("wo", [128, 8, 1024], BF16, kind="ExternalInput").ap()
    out_d = nc.dram_tensor("out", [4, 128, 1024], F32, kind="ExternalOutput").ap()

    with tile.TileContext(nc) as tc:
        with (
            tc.tile_pool(name="keep", bufs=1) as keep,
            tc.tile_pool(name="exp", bufs=8) as expp,
            tc.tile_pool(name="rcp", bufs=3) as rcpp,
            tc.tile_pool(name="osb", bufs=2) as osbp,
            tc.tile_pool(name="ps", bufs=1, space="PSUM") as psp,
        ):
            # ------- input DMAs: single-shot weights, streamed xt chunks -----
            # sync/scalar are hardware-DGE queues (fast); gpsimd is software-
            # DGE (slow) and only carries xt7 plus small latency-tolerant
            # intra-attention copies.  Startup-critical prefix: pair-0 slices
            # of Wq/Wk (pair-1 isn't read until the attn0-qc1 fillers), wv in
            # two kt-halves, and the seq-low halves of xt (the pre-attention
            # block only reads seq columns 0:1024) - ~3MB instead of 5.9MB.
            wqt = keep.tile([128, 8, 256], F16, tag="wq", name="wq")
            wkt = keep.tile([128, 8, 256], F16, tag="wk", name="wk")
            wvt = keep.tile([128, 8, 256], F16, tag="wv", name="wv")
            wot = keep.tile([128, 8, 1024], BF16, tag="wo", name="wo")
            xt = [keep.tile([128, 2048], F16, tag=f"xt{kt}", name=f"xt{kt}")
                  for kt in range(8)]
            nc.sync.dma_start(out=wqt[:, :, 0:128], in_=wq_d[:, :, 0:128])
            nc.scalar.dma_start(out=xt[0][:, 0:1024], in_=xt_d[:, 0, 0:1024])
            nc.sync.dma_start(out=wvt[:, 0:4, :], in_=wv_d[:, 0:4, :])
            nc.scalar.dma_start(out=wkt[:, :, 0:128], in_=wk_d[:, :, 0:128])
            nc.gpsimd.dma_start(out=xt[7][:, 0:1024], in_=xt_d[:, 7, 0:1024])
            for kt in range(1, 7):
                eng = (nc.scalar, nc.sync)[kt % 2]
                eng.dma_start(out=xt[kt][:, 0:1024], in_=xt_d[:, kt, 0:1024])
            nc.scalar.dma_start(out=wvt[:, 4:8, :], in_=wv_d[:, 4:8, :])
            for kt in range(7):
                eng = (nc.sync, nc.scalar)[kt % 2]
                eng.dma_start(out=xt[kt][:, 1024:2048], in_=xt_d[:, kt, 1024:2048])
            nc.gpsimd.dma_start(out=xt[7][:, 1024:2048], in_=xt_d[:, 7, 1024:2048])
            nc.scalar.dma_start(out=wqt[:, :, 128:256], in_=wq_d[:, :, 128:256])
            nc.sync.dma_start(out=wkt[:, :, 128:256], in_=wk_d[:, :, 128:256])
            nc.sync.dma_start(out=wot[:], in_=wo_d[:])
            wq = [wqt[:, kt, :] for kt in range(8)]
            wk = [wkt[:, kt, :] for kt in range(8)]
            wv = [wvt[:, kt, :] for kt in range(8)]
            wo = [wot[:, kt, :] for kt in range(8)]

            # q2b[p, hg, r, jl]: p>=64 holds j=8+jl qkv, p<64 holds j=jl
            # (DMA-shifted from stg); Wo is repacked to match: contraction
            # chunk c pairs Wo rows [64c,64c+64) with [512+64c, 512+64c+64).
            q2b = keep.tile([128, 4, 128, 8], BF16, tag="q2b")
            stg = keep.tile([128, 4, 128, 8], BF16, tag="stg")

            # ------- persistent result tiles -------
            v_sb = {}
            for kb in range(16):
                vt = keep.tile([128, 4, 128], BF16, tag=f"v{kb}", name=f"v{kb}")
                v_sb[kb] = vt
                nc.gpsimd.memset(vt[:], 1.0)
            # PE warm-up: ~14 dummy matmuls over the ones-initialized v0
            # tile during the DMA-bound startup window (PE would idle
            # anyway); keeps the HAM activity window hot so the first real
            # matmuls run at 2.4GHz instead of the cold 1.2GHz p-state.
            # Result accumulates in a scratch av-tag bank, never read.
            warm = psp.tile([128, 512], F32, tag="av", bufs=4, name="warm")
            v0flat = v_sb[0][:].rearrange("p a b -> p (a b)")
            for i in range(17):
                nc.tensor.matmul(
                    warm[:], v_sb[0][:, 0, :], v0flat[:, 0:512],
                    start=(i == 0), stop=(i == 16),
                )

            qk_sb = {}
            for pair in range(2):
                qk_sb[("q", pair)] = keep.tile(
                    [128, 2048], F16, tag=f"qT{pair}", name=f"qT{pair}")
                qk_sb[("k", pair)] = keep.tile(
                    [128, 2048], F16, tag=f"kT{pair}", name=f"kT{pair}")

            # ------- emit helpers -------
            def v_mm(kb, ps, kt):
                nc.tensor.matmul(
                    ps[:],
                    xt[kt][:, kb * 128:(kb + 1) * 128],
                    wv[kt],
                    start=(kt == 0),
                    stop=(kt == 7),
                )

            def v_copy(kb, ps):
                nc.vector.tensor_copy(
                    v_sb[kb][:, :, 64:128],
                    ps[:].rearrange("p (a b) -> p a b", a=4))

            def v_group(kb, tag="sc", bufs=2):
                ps = psp.tile([128, 256], F32, tag=tag, bufs=bufs, name="vps")
                for kt in range(8):
                    v_mm(kb, ps, kt)
                v_copy(kb, ps)

            def qk_mm(nm, pair, ch, ps, kt):
                wt = wqt if nm == "q" else wkt
                nc.tensor.matmul(
                    ps[:],
                    wt[:, kt, pair * 128:(pair + 1) * 128],
                    xt[kt][:, ch * 512:(ch + 1) * 512],
                    start=(kt == 0),
                    stop=(kt == 7),
                )

            def qk_copy(nm, pair, ch, ps):
                cs = slice(ch * 512, (ch + 1) * 512)
                nc.vector.tensor_copy(qk_sb[(nm, pair)][:, cs], ps[:])

            def qk_group(nm, pair, ch):
                ps = psp.tile([128, 512], F32, tag="sc", bufs=2, name="qkps")
                for kt in range(8):
                    qk_mm(nm, pair, ch, ps, kt)
                qk_copy(nm, pair, ch, ps)

            def oproj_half(hg, h, tag="sc", bufs=2, out_eng=None):
                # one 512-wide half of the O-projection for head hg; sized to
                # hold a single PSUM buf so it can run as an attention filler
                ops = psp.tile([128, 512], F32, tag=tag, bufs=bufs, name="ops")
                for kt in range(8):
                    nc.tensor.matmul(
                        ops[:],
                        q2b[:, hg, :, kt],
                        wot[:, kt, h * 512:(h + 1) * 512],
                        start=(kt == 0),
                        stop=(kt == 7),
                    )
                ot = osbp.tile([128, 512], F32, tag="ot", name="ot")
                nc.vector.tensor_copy(ot[:], ops[:])
                if out_eng is None:
                    nc.sync.dma_start(
                        out=out_d[hg, :, h * 512:(h + 1) * 512], in_=ot[:])
                else:
                    # tail: quarter-pieces across both HWDGE queues so the
                    # final transfer overlaps the final copies
                    for piece in range(2):
                        cs = slice(h * 512 + piece * 256, h * 512 + piece * 256 + 256)
                        eng = (nc.sync, nc.scalar)[piece]
                        eng.dma_start(out=out_d[hg, :, cs],
                                      in_=ot[:, piece * 256:piece * 256 + 256])

            def attn(pair, fillers):
                qT = qk_sb[("q", pair)]
                kT = qk_sb[("k", pair)]
                fi = 0
                it = 0
                for qc in range(2):
                    av = {}
                    for hl in range(2):
                        for half in range(2):
                            av[(hl, half)] = psp.tile(
                                [128, 512], F32, tag="av", bufs=4, name="av")

                    def av_mms(kbp, et_prev, hls=(0, 1)):
                        for hl in hls:
                            hg = 2 * pair + hl
                            lhsT = v_sb[kbp][:, hg, :]
                            for half in range(2):
                                nc.tensor.matmul(
                                    av[(hl, half)][:],
                                    lhsT,
                                    et_prev[hl][:, half * 512:(half + 1) * 512],
                                    start=(kbp == 0),
                                    stop=(kbp == 15),
                                )

                    prev_et = None
                    for kb in range(16):
                        while fi < len(fillers) and fillers[fi][0] <= it:
                            fillers[fi][1]()
                            fi += 1
                        it += 1
                        sc = {}
                        for hl in range(2):
                            sc[hl] = psp.tile(
                                [128, 1024], F32, tag="sc", bufs=2, name="sc")
                        # K=64 row-tiled score matmuls: hl0 (rows 0:64) and
                        # hl1 (rows 64:128) issued adjacently run concurrently
                        for sub in range(2):
                            q0 = qc * 1024 + sub * 512
                            for hl in range(2):
                                rows = slice(64 * hl, 64 * hl + 64)
                                nc.tensor.matmul(
                                    sc[hl][:, sub * 512:(sub + 1) * 512],
                                    kT[rows, kb * 128:(kb + 1) * 128],
                                    qT[rows, q0:q0 + 512],
                                    start=True,
                                    stop=True,
                                )
                        # software-pipelined AV: consume exp of kb-1 so the
                        # PE never waits on ScalarE inside an iteration
                        if prev_et is not None:
                            av_mms(kb - 1, prev_et)
                        et = {}
                        for hl in range(2):
                            et[hl] = expp.tile([128, 1024], BF16, tag="exp", name="et")
                            nc.scalar.activation(et[hl][:], sc[hl][:], EXP)
                        prev_et = et
                    # interleave the final AV matmuls per head with the DVE
                    # drain: hl0's reciprocals run while the PE finishes hl1
                    rts = {}
                    def drain_rcp(hl):
                        for half in range(2):
                            rt = rcpp.tile([128, 512], F32, tag="rcp", bufs=4,
                                           name="rt")
                            rts[(hl, half)] = rt
                            nc.vector.reciprocal_approx_fast(
                                rt[0:64, :], av[(hl, half)][0:64, :])
                            beng = nc.sync if half == 0 else nc.scalar
                            beng.dma_start(out=rt[64:128, :], in_=rt[0:64, :])
                    av_mms(15, prev_et, hls=(0,))
                    drain_rcp(0)
                    av_mms(15, prev_et, hls=(1,))
                    drain_rcp(1)
                    for hl in range(2):
                        hg = 2 * pair + hl
                        for half in range(2):
                            ap = av[(hl, half)]
                            rt = rts[(hl, half)]
                            u0 = qc * 64 + half * 32
                            apv = ap[64:128, :].rearrange("p (r j) -> p r j", j=16)
                            rtv = rt[64:128, :].rearrange("p (r j) -> p r j", j=16)
                            nc.vector.tensor_mul(
                                q2b[64:128, hg, u0:u0 + 32, :],
                                apv[:, :, 8:16], rtv[:, :, 8:16])
                            nc.vector.tensor_mul(
                                stg[64:128, hg, u0:u0 + 32, :],
                                apv[:, :, 0:8], rtv[:, :, 0:8])
                        # shift this qc's r-half of the low-j qkv into
                        # partitions 0:64 as soon as the head's muls land
                        # (contiguous DMA; scalar queue is parallel to the
                        # rcp broadcasts on sync)
                        r0 = qc * 64
                        nc.scalar.dma_start(
                            out=q2b[0:64, hg, r0:r0 + 64, :],
                            in_=stg[64:128, hg, r0:r0 + 64, :],
                        )
                while fi < len(fillers):
                    fillers[fi][1]()
                    fi += 1

            # ------- pre-attention block: qT0 ch0/ch1 + kT0 ch0 kt-major so
            # their matmuls chase the streaming xt DMA arrivals, then V0..V7
            # group-major to fill the rest of the DMA-bound startup window
            pre_ps = {
                ("q", 0): psp.tile([128, 512], F32, tag="sc", bufs=2, name="pq0"),
                ("q", 1): psp.tile([128, 512], F32, tag="sc", bufs=2, name="pq1"),
                ("k", 0): psp.tile([128, 512], F32, tag="av", bufs=4, name="pk0"),
            }
            pre_v = {kb: psp.tile([128, 256], F32, tag="av", bufs=4,
                                  name=f"pv{kb}") for kb in range(3)}
            for kt in range(8):
                qk_mm("q", 0, 0, pre_ps[("q", 0)], kt)
                qk_mm("q", 0, 1, pre_ps[("q", 1)], kt)
                qk_mm("k", 0, 0, pre_ps[("k", 0)], kt)
                for kb in range(3):
                    v_mm(kb, pre_v[kb], kt)
            qk_copy("q", 0, 0, pre_ps[("q", 0)])
            qk_copy("q", 0, 1, pre_ps[("q", 1)])
            qk_copy("k", 0, 0, pre_ps[("k", 0)])
            for kb in range(3):
                v_copy(kb, pre_v[kb])
            for kb in range(3, 8):
                v_group(kb)

            # fillers for attn0: remaining V just in time, k0 chunks before
            # their kb range, q0 qc1 chunks before slot 16, attn1 prework
            f0 = []
            f0.append((0, lambda: v_group(8)))
            f0.append((1, lambda: qk_group("k", 0, 1)))
            f0.append((2, lambda: v_group(9)))
            f0.append((3, lambda: v_group(10)))
            f0.append((4, lambda: v_group(11)))
            f0.append((5, lambda: qk_group("k", 0, 2)))
            f0.append((6, lambda: v_group(12)))
            f0.append((7, lambda: v_group(13)))
            f0.append((8, lambda: v_group(14)))
            f0.append((9, lambda: qk_group("k", 0, 3)))
            f0.append((10, lambda: v_group(15)))
            f0.append((11, lambda: qk_group("q", 0, 2)))
            f0.append((13, lambda: qk_group("q", 0, 3)))
            f0.append((16, lambda: qk_group("k", 1, 0)))
            f0.append((19, lambda: qk_group("q", 1, 0)))
            f0.append((22, lambda: qk_group("q", 1, 1)))
            f0.append((26, lambda: qk_group("k", 1, 1)))
            attn(0, f0)

            # fillers for attn1: remaining k1/q1 chunks ahead of their use,
            # plus the pair-0 O-projection halves (q2b hg0/hg1 are complete)
            f1 = []
            f1.append((2, lambda: qk_group("k", 1, 2)))
            f1.append((6, lambda: qk_group("k", 1, 3)))
            f1.append((10, lambda: qk_group("q", 1, 2)))
            f1.append((13, lambda: qk_group("q", 1, 3)))
            f1.append((16, lambda: oproj_half(0, 0)))
            f1.append((99, lambda: oproj_half(0, 1)))
            f1.append((99, lambda: oproj_half(1, 0)))
            # hg1's second half trails the loop: its matmuls cover the PE
            # while the tail rcp/mul/shift chain for hg2/hg3 drains
            f1.append((99, lambda: oproj_half(1, 1)))
            attn(1, f1)

            # keep the PE hot across the tail drain chain (rcp/mul/shift);
            # these dependency-free dummies soak the otherwise-idle ~2us so
            # HAM doesn't re-throttle right before the tail projections
            wtail = psp.tile([128, 512], F32, tag="av", bufs=4, name="wtail")
            for i in range(9):
                nc.tensor.matmul(
                    wtail[:], v_sb[0][:, 0, :], v0flat[:, 0:512],
                    start=(i == 0), stop=(i == 8),
                )

            # tail: pair-1 output projections on the freed av-tag banks;
            # out-DMAs split across both HWDGE queues (scalar is free now)
            for hg in (2, 3):
                for h in range(2):
                    oproj_half(hg, h, tag="av", bufs=4,
                               out_eng=nc.sync if h == 0 else nc.scalar)

    nc.compile()
    return nc


def _get_program():
    if "nc" not in _CACHE:
        _CACHE["nc"] = _build_program()
    return _CACHE["nc"]


def _make_in_maps(x, Wq, Wk, Wv, Wo):
    import ml_dtypes

    bf16 = ml_dtypes.bfloat16
    # chunk c pairs Wo rows [64c, 64c+64) (partitions 0:64) with rows
    # [512+64c, 512+64c+64) (partitions 64:128) to match the q2b layout
    wo8 = np.ascontiguousarray(
        Wo.astype(bf16).reshape(2, 8, 64, 1024).transpose(0, 2, 1, 3).reshape(
            128, 8, 1024))
    xts = [
        np.ascontiguousarray(
            x[b].T.astype(np.float16).reshape(8, 128, 2048).transpose(1, 0, 2))
        for b in range(B)
    ]
    wq16 = Wq.astype(np.float16)
    wk16 = Wk.astype(np.float16)
    wv16 = Wv.astype(np.float16)
    def pack(w, cols):
        return np.ascontiguousarray(
            w[:, cols].reshape(8, 128, 256).transpose(1, 0, 2))
    in_maps = []
    for c in range(NCORES):
        b, g = c // 4, c % 4
        cols = slice(4 * g * DH, 4 * (g + 1) * DH)
        in_maps.append(
            {
                "xt": xts[b],
                "wq": pack(wq16, cols),
                "wk": pack(wk16, cols),
                "wv": pack(wv16, cols),
                "wo": wo8,
            }
        )
    return in_maps


def kernel(x, Wq, Wk, Wv, Wo, n_head):
    from concourse.bass_utils import run_bass_kernel_spmd

    assert int(n_head) == H
    x = np.asarray(x, np.float32)
    Wq = np.asarray(Wq, np.float32)
    Wk = np.asarray(Wk, np.float32)
    Wv = np.asarray(Wv, np.float32)
    Wo = np.asarray(Wo, np.float32)

    nc = _get_program()
    in_maps = _make_in_maps(x, Wq, Wk, Wv, Wo)
    res = run_bass_kernel_spmd(nc, in_maps, list(range(NCORES)))

    out = np.empty((B, S, D), np.float32)
    for c in range(NCORES):
        b, g = c // 4, c % 4
        out[b, g * 512:(g + 1) * 512, :] = np.asarray(
            res.results[c]["out"], np.float32).reshape(512, 1024)
    return out
